# revision 52
# baseline (speedup 1.0000x reference)
"""Cross-frame attention kernel for 8 TRN2 NeuronCores.

Sharding: core c handles batch b = c//2 and head-group g = c%2 (4 of the 8
heads).  The host pre-transposes x[b]/context[b] (feature dim onto SBUF
partitions) and casts to bf16; each core computes a partial output
(its 4 heads pushed through the matching Wo rows) and the host sums the
two partials per batch plus the bias.

Device math per core (S^T layout, softmax over the partition j-dim):
  QT = Wq_g^T x^T          [256, 2048]
  KT = Wk_g^T c^T          [256, 2048]
  V  = c Wv_g              [2048, 256] (+ ones column per head)
  S^T = K_h Q_h^T          [j, i] tiles; j-tiles 0..5 run as fp8e4
                           DoubleRow matmuls (2x PE rate, 32+32 d-subtile
                           pairing), j-tiles 6..15 in bf16
  exp via ScalarE (scale=1/8 fused), bf16 out
  O~^T | Z = [V_h|1]^T expS^T   (PSUM accumulate over j)
  A^T = O~^T * bcast(1/Z)  (reciprocal on DVE, partition broadcast on
                           GpSimd/Pool, multiply on DVE)
  out_partial = A^T^T Wo_g [2048, 512] fp32

The schedule is a flat software pipeline over 128 (block, j-group) items:
per item the PE issues the S matmuls first (so the ScalarE exp stream is
never gated by downstream work), then projection / V / normalization /
output-projection fillers, then the AV matmuls of the item two groups
back.  The per-block normalization runs on DVE+Pool only, keeping PE free.

The exp stream on the Scalar engine (16.8M exps/core at ~0.83ns/col) is
the roofline for this kernel; PE is kept just below it via the partial
fp8 S.  fp8 Q/K quantization (after bf16 projection; DVE PSUM->fp8 copy
plus a small SBUF->SBUF DMA shuffle into the [32, 2-dslab, head, seq]
DoubleRow layout) perturbs outputs by ~1.1e-2 rel on 6/16 of the j-range,
inside the 2e-2 tolerance.

Logits are |S/8| <~ 1.1 for this problem's scale, so softmax without
max-subtraction is exact in fp32.
"""

import numpy as np
import ml_dtypes

B = 4
N = 2048  # query length
M = 2048  # context length
DIM = 512
HEADS = 8
DH = 64
HC = 256  # head columns handled per core (4 heads)
P = 128
KO = DIM // P  # 4 k-chunks
NI4 = N // 512  # 4 i-chunks of 512
NJ = M // P  # 16 j-chunks
JPG = 2  # j-chunks per exp group (PSUM banks per S^T buffer)
NG = NJ // JPG  # 8 j-groups per block
FP8_GROUPS = (0, 1, 2, 3)  # j-groups using fp8 DoubleRow (j-tiles 0..7)

_CACHE = {}


def _build():
    from contextlib import ExitStack

    import concourse.mybir as mybir
    import concourse.tile as tile
    from concourse import bacc

    bf = mybir.dt.bfloat16
    f32 = mybir.dt.float32
    fp8 = mybir.dt.float8e4
    Exp = mybir.ActivationFunctionType.Exp
    DR = mybir.MatmulPerfMode.DoubleRow

    nc = bacc.Bacc(None, target_bir_lowering=False, debug=False)
    with tile.TileContext(nc) as tc:
        with ExitStack() as ctx:
            dram = ctx.enter_context(tc.tile_pool(name="dram", bufs=1, space="DRAM"))
            xT_d = dram.tile([DIM, N], bf, kind="ExternalInput")
            cT_d = dram.tile([DIM, M], bf, kind="ExternalInput")
            wq_d = dram.tile([DIM, HC], bf, kind="ExternalInput")
            wk_d = dram.tile([DIM, HC], bf, kind="ExternalInput")
            wv_d = dram.tile([DIM, HC], bf, kind="ExternalInput")
            wo_d = dram.tile([HC, DIM], bf, kind="ExternalInput")
            out_d = dram.tile([N, DIM], f32, kind="ExternalOutput")

            const = ctx.enter_context(tc.tile_pool(name="const", bufs=1))

            xt_sb = const.tile([P, KO, N], bf, tag="xt")
            ct_sb = const.tile([P, KO, M], bf, tag="ct")
            wq_sb = const.tile([P, KO, HC], bf, tag="wq")
            wk_sb = const.tile([P, KO, HC], bf, tag="wk")
            wv_sb = const.tile([P, KO, HC], bf, tag="wv")
            wo_sb = const.tile([P, 2, DIM], bf, tag="wo")
            qT_sb = const.tile([P, 2, N], bf, tag="qT")
            kT_sb = const.tile([P, 2, M], bf, tag="kT")
            # fp8 staging (projection layout) + DoubleRow layout
            # [dmod32, dslab, head, seq]; k8 covers all of M for m=0 (blocks
            # 1-3 run fully fp8) but only j-tiles 0..5 for m=1
            qs8_sb = const.tile([P, 2, N], fp8, tag="qs8")
            ks8_sb = const.tile([P, 2, M], fp8, tag="ks8")
            q8_sb = const.tile([32, 2, 4, N], fp8, tag="q8")
            k8_sb = const.tile([32, 2, 4, M], fp8, tag="k8")
            warm_sb = const.tile([1, 512], bf, tag="warm")
            # all 4 heads' V with a trailing ones column: [j, jo, head, 65]
            vp_sb = const.tile([P, NJ, 4, DH + 1], bf, tag="vp")
            aT_sb = const.tile([P, 2, N], bf, tag="aT")

            dummy_sb = const.tile([1, 1], f32, tag="dummy")
            nc.vector.memset(vp_sb[:, :, :, DH : DH + 1], 1.0)
            nc.vector.memset(warm_sb[:], 0.0)
            # hoist the exp ACT-table load out of the critical path
            nc.scalar.activation(dummy_sb[:], vp_sb[0:1, 0, 0, DH : DH + 1],
                                 Exp, scale=1.0)

            # DMA in, first-needed first, ONE dma_start per piece: the HWDGE
            # queue dispatches serially at ~625ns/DMA, so consolidated
            # transfers keep the preamble's shuffle DMAs near the queue head.
            # The remaining input pieces are issued after the preamble (see
            # below) so they queue behind its shuffle DMAs.
            cT_r = cT_d[:].rearrange("(ko p) i -> p ko i", p=P)
            xT_r = xT_d[:].rearrange("(ko p) i -> p ko i", p=P)
            nc.sync.dma_start(wk_sb[:], wk_d[:].rearrange("(ko p) m -> p ko m", p=P))
            nc.sync.dma_start(ct_sb[:, :, 0:512], cT_r[:, :, 0:512])
            nc.sync.dma_start(wq_sb[:], wq_d[:].rearrange("(ko p) m -> p ko m", p=P))
            nc.sync.dma_start(xt_sb[:, :, 0:512], xT_r[:, :, 0:512])
            nc.sync.dma_start(ct_sb[:, :, 512:1024], cT_r[:, :, 512:1024])
            nc.sync.dma_start(wv_sb[:], wv_d[:].rearrange("(ko p) m -> p ko m", p=P))
            nc.sync.dma_start(ct_sb[:, :, 1024:1536], cT_r[:, :, 1024:1536])
            nc.sync.dma_start(ct_sb[:, :, 1536:2048], cT_r[:, :, 1536:2048])
            nc.sync.dma_start(xt_sb[:, :, 512:1024], xT_r[:, :, 512:1024])

            def dma_xt23_wo():
                nc.sync.dma_start(xt_sb[:, :, 1024:2048], xT_r[:, :, 1024:2048])
                nc.sync.dma_start(
                    wo_sb[:], wo_d[:].rearrange("(r p) n -> p r n", p=P))

            # PSUM budget (8 banks): s 2x2 + o 2x1 + aux 1 + scr 1
            with (
                tc.tile_pool(name="s_ps", bufs=2, space="PSUM") as s_pool,
                tc.tile_pool(name="aux_ps", bufs=1, space="PSUM") as aux_pool,
                tc.tile_pool(name="o_ps", bufs=2, space="PSUM") as o_pool,
                tc.tile_pool(name="scr_ps", bufs=1, space="PSUM") as scr_pool,
                tc.tile_pool(name="e_sb", bufs=14) as e_pool,
                tc.tile_pool(name="small", bufs=2) as small,
                tc.tile_pool(name="ost", bufs=4) as ostp,
            ):
                # ---- building blocks -------------------------------------
                def qk_chunk_pieces(wsb, src_sb, stage8, dst8, bfdst, m, c,
                                    want8, want_bf, pool_tag):
                    """Filler pieces: A/B = 2 ko-matmuls each (B also does
                    the bf16 copy), C = fp8 quantize + DoubleRow shuffle.
                    A chunk's pieces must stay consecutive w.r.t. other
                    tenants of the same PSUM bank."""
                    isl = slice(c * 512, (c + 1) * 512)
                    pool, tag = pool_tag
                    box = []

                    def piece_a():
                        ps = pool.tile([P, 512], f32, tag=tag, name="ps_qk")
                        box.append(ps)
                        for ko in (0, 1):
                            nc.tensor.matmul(
                                ps[:], wsb[:, ko, m * P : (m + 1) * P],
                                src_sb[:, ko, isl],
                                start=(ko == 0), stop=False)

                    def piece_b():
                        ps = box[0]
                        for ko in (2, 3):
                            nc.tensor.matmul(
                                ps[:], wsb[:, ko, m * P : (m + 1) * P],
                                src_sb[:, ko, isl],
                                start=False, stop=(ko == 3))
                        if want_bf:
                            nc.vector.tensor_copy(bfdst[:, m, isl], ps[:])

                    def piece_c():
                        # quantize from the bf16 SBUF copy when available:
                        # the PSUM bank then frees at piece_b, so the next
                        # bank tenant's matmuls don't wait on this copy
                        src = bfdst[:, m, isl] if want_bf else box[0][:]
                        nc.vector.tensor_copy(stage8[:, m, isl], src)
                        for hl in range(2):
                            for s in range(2):
                                pb8 = 64 * hl + 32 * s
                                nc.sync.dma_start(
                                    dst8[:, s, 2 * m + hl, isl],
                                    stage8[pb8 : pb8 + 32, m, isl],
                                )

                    if want8:
                        return piece_a, piece_b, piece_c
                    return piece_a, piece_b

                def kchunk(m, c):
                    # chunk 0 also needs bf16 for m=0 (block 0 runs all-bf16
                    # so its start doesn't wait on the fp8 shuffle DMAs)
                    return qk_chunk_pieces(
                        wk_sb, ct_sb, ks8_sb, k8_sb, kT_sb, m, c,
                        want8=(m == 0 or c < 2), want_bf=(c >= 1 or m == 0),
                        pool_tag=(aux_pool, "aux"))

                def qchunk(m, c):
                    # q-chunks use the scr bank so their PSUM tenancy never
                    # interleaves with k-chunks/vpairs on the aux bank
                    return qk_chunk_pieces(
                        wq_sb, xt_sb, qs8_sb, q8_sb, qT_sb, m, c,
                        want8=True, want_bf=True,
                        pool_tag=(scr_pool, "scr"))

                def vpair_half(g, half):
                    def f():
                        jo = 2 * g + half
                        # alternate banks so each half's matmuls overlap the
                        # previous half's PSUM->SBUF copy instead of WAR-
                        # stalling on the single bank
                        pool, tag = ((aux_pool, "aux") if jo % 2 == 0
                                     else (scr_pool, "scr"))
                        ps = pool.tile([P, HC], f32, tag=tag, name="ps_v")
                        for ko in range(KO):
                            nc.tensor.matmul(
                                ps[:], ct_sb[:, ko, jo * P : (jo + 1) * P],
                                wv_sb[:, ko, :],
                                start=(ko == 0), stop=(ko == KO - 1))
                        nc.vector.tensor_copy(
                            vp_sb[:, jo, :, 0:DH],
                            ps[:].rearrange("p (h d) -> p h d", h=4))
                    return f

                o_tiles = {}
                e_tiles = {}

                def emit_S(bidx, blk, g):
                    i4, m, hl = blk
                    h = 2 * m + hl
                    isl = slice(i4 * 512, (i4 + 1) * 512)
                    s_ps = s_pool.tile([P, JPG, 512], f32, tag="s", name="s_ps")
                    # uniform fp8 fraction across (i, h) so the absmax error
                    # stays even; block 0 g0 stays bf16 (its fp8 staging
                    # isn't shuffled yet at stream start)
                    fp8_here = g in FP8_GROUPS and bidx > 0
                    for jj in range(JPG):
                        j = g * JPG + jj
                        if fp8_here:
                            nc.tensor.matmul(
                                s_ps[:, jj, :],
                                k8_sb[:, :, h, j * P : (j + 1) * P],
                                q8_sb[:, :, h, isl],
                                start=True, stop=True, perf_mode=DR)
                        else:
                            pb = DH * hl
                            nc.tensor.matmul(
                                s_ps[:, jj, :],
                                kT_sb[pb : pb + DH, m, j * P : (j + 1) * P],
                                qT_sb[pb : pb + DH, m, isl],
                                start=True, stop=True)
                    e_sb = e_pool.tile([P, JPG, 512], bf, tag="e", name="e_sb")
                    nc.scalar.activation(e_sb[:], s_ps[:], Exp, scale=0.125)
                    e_tiles[(bidx, g)] = e_sb

                def emit_AV(bidx, blk, g):
                    i4, m, hl = blk
                    h = 2 * m + hl
                    if g == 0:
                        o_tiles[bidx] = o_pool.tile(
                            [DH + 1, 512], f32, tag="o", name="o_ps")
                    o_ps = o_tiles[bidx]
                    e_sb = e_tiles.pop((bidx, g))
                    for jj in range(JPG):
                        j = g * JPG + jj
                        nc.tensor.matmul(
                            o_ps[:], vp_sb[:, j, h, :], e_sb[:, jj, :],
                            start=(j == 0), stop=(j == NJ - 1))

                def finish(bidx, blk):
                    def f():
                        i4, m, hl = blk
                        pb = DH * hl
                        isl = slice(i4 * 512, (i4 + 1) * 512)
                        o_ps = o_tiles.pop(bidx)
                        rz = small.tile([1, 512], f32, tag="rz", name="rz")
                        nc.vector.reciprocal(rz[:], o_ps[DH : DH + 1, :])
                        rbc = small.tile([DH, 512], f32, tag="rbc", name="rbc")
                        nc.gpsimd.partition_broadcast(rbc[:], rz[0:1, :])
                        nc.vector.tensor_mul(
                            aT_sb[pb : pb + DH, m, isl], o_ps[0:DH, :], rbc[:])
                    return f

                def wo_piece(i4, ii, tail=False):
                    def f():
                        i = i4 * 4 + ii
                        pool, tag = ((scr_pool, "scr") if ii % 2 == 0
                                     else (aux_pool, "aux"))
                        ps = pool.tile([P, DIM], f32, tag=tag, name="p3_ps")
                        for m in range(2):
                            nc.tensor.matmul(
                                ps[:], aT_sb[:, m, i * P : (i + 1) * P],
                                wo_sb[:, m, :],
                                start=(m == 0), stop=(m == 1))
                        ost = ostp.tile([P, DIM], f32, tag="ost", name="ost")
                        if tail and ii % 2 == 1:
                            # post-stream: ScalarE is idle, split the copies
                            nc.scalar.copy(ost[:], ps[:])
                        else:
                            nc.vector.tensor_copy(ost[:], ps[:])
                        nc.sync.dma_start(out_d[i * P : (i + 1) * P, :], ost[:])
                    return f

                # ---- schedule --------------------------------------------
                blocks = [
                    (0, 0, 0), (0, 0, 1), (1, 0, 0), (1, 0, 1),
                    (0, 1, 0), (0, 1, 1), (2, 0, 0), (2, 0, 1),
                    (1, 1, 0), (1, 1, 1), (3, 0, 0), (3, 0, 1),
                    (2, 1, 0), (2, 1, 1), (3, 1, 0), (3, 1, 1),
                ]

                fillers = {}

                def add(bidx, g, *fs):
                    fillers.setdefault((bidx, g), []).extend(fs)

                # block 0 j-groups run all-bf16 and its AVs lag 8 groups, so
                # the 8 V-pair projections spread over blocks 0-1.  All aux-
                # bank tenants (k-chunks, vpairs) are laid out strictly
                # sequentially: a tenant's pieces finish before the next
                # tenant's first piece.
                add(0, 0, *kchunk(0, 1))            # bf16 by S(0,2); fp8 by blk1 g2
                add(0, 2, *kchunk(0, 2))            # by S(0,4)
                add(0, 4, *kchunk(0, 3))            # by S(0,6)
                for k in range(8):
                    b_at, g_at = divmod(7 + k, NG)  # vp0 (0,7) ... vp7 (1,6)
                    add(b_at, g_at, vpair_half(k, 0), vpair_half(k, 1))
                for (m, c), (b_at, g_at) in {       # aux tenants, sequential
                    (1, 0): (2, 0),   # k8 m=1 used from block 4 (fp8)
                    (1, 1): (2, 2),
                    (1, 2): (3, 2),   # kT bf16 m=1 groups 4+
                    (1, 3): (3, 4),
                }.items():
                    add(b_at, g_at, *kchunk(m, c))
                for (m, c), (b_at, g_at) in {       # scr tenants, sequential
                    (0, 1): (0, 5),   # qT i4=1 used from block 2
                    (1, 0): (3, 0),   # q m=1 used from block 4
                    (0, 2): (4, 0),   # i4=2 used from block 6
                    (1, 1): (5, 0),   # m=1 i4=1 used from block 8
                    (0, 3): (7, 0),   # i4=3 used from block 10
                    (1, 2): (9, 0),   # used from block 12
                    (1, 3): (11, 0),  # used from block 14
                }.items():
                    pieces = qchunk(m, c)
                    add(b_at, g_at, pieces[0])
                    add(b_at, g_at + 1, *pieces[1:])
                # The AV lag stays at 8 through the first three blocks
                # (absorbing the V-projection PE deficit) and then decays to
                # the steady-state 2; the last block drops to 1 so the tail
                # drains fast.
                def lag_at(gidx, bidx):
                    if bidx == 15:
                        return 1
                    if gidx < 32:
                        return 12
                    return max(3, 12 - (gidx - 31) // 3)

                # dry-run the lag queue to learn when each AV is emitted,
                # then place each block's normalization right after its last
                # AV and each wo after its four blocks' normalizations
                pop_slot = {}
                dq = []
                for b in range(16):
                    for g in range(NG):
                        gidx = b * NG + g
                        dq.append((b, g))
                        while len(dq) > lag_at(gidx, b):
                            pop_slot[dq.pop(0)] = gidx
                for item in dq:
                    pop_slot[item] = 16 * NG
                fin_slot = {}
                for b in range(16):
                    fin_slot[b] = pop_slot[(b, 7)] + 1
                    if fin_slot[b] < 16 * NG:
                        add(*divmod(fin_slot[b], NG), finish(b, blocks[b]))
                for i4 in range(4):
                    s0 = max(fin_slot[b] for b, blk in enumerate(blocks)
                             if blk[0] == i4) + 1
                    if s0 + 6 < 16 * NG:
                        for ii in range(4):
                            add(*divmod(s0 + 2 * ii, NG), wo_piece(i4, ii))

                # PE warm-up: keep the tensor engine busy from t~0.7us so
                # the p-state ramp (full speed only after 3us of continuous
                # execution) completes before the first real projections.
                warm_ps = aux_pool.tile([P, 512], f32, tag="aux", name="warm")
                for _ in range(9):
                    nc.tensor.matmul(warm_ps[0:1, :], warm_sb[0:1, 0:1],
                                     warm_sb[:], start=True, stop=True)

                # preamble: the chunk-0 projections feeding the first block
                # (k on aux, q on scr so they overlap), fp8 staging last so
                # the bf16 copies feeding S(0,0) finish first; remaining
                # input DMAs queue behind the preamble shuffle DMAs.
                k00 = kchunk(0, 0)
                q00 = qchunk(0, 0)
                k00[0](); k00[1]()
                q00[0](); q00[1]()
                k00[2](); q00[2]()
                # bulk input DMAs are emitted as late fillers so the early
                # fp8-shuffle DMAs aren't stuck behind them in the queue
                add(1, 7, dma_xt23_wo)

                # main loop: S first, then fillers, then lagged AV; the AV
                # lag ramps 8 -> 2 across blocks 1-2 (absorbing block 0's
                # V-projection PE deficit) and drops to 1 for the last block
                av_q = []
                for bidx, blk in enumerate(blocks):
                    for g in range(NG):
                        gidx = bidx * NG + g
                        emit_S(bidx, blk, g)
                        for f in fillers.get((bidx, g), []):
                            f()
                        av_q.append((bidx, blk, g))
                        while len(av_q) > lag_at(gidx, bidx):
                            emit_AV(*av_q.pop(0))
                while av_q:
                    emit_AV(*av_q.pop(0))
                for b in range(16):
                    if fin_slot[b] >= 16 * NG:
                        finish(b, blocks[b])()
                for i4 in range(4):
                    s0 = max(fin_slot[b] for b, blk in enumerate(blocks)
                             if blk[0] == i4) + 1
                    if s0 + 3 >= 16 * NG:
                        for ii in range(4):
                            wo_piece(i4, ii, tail=True)()

    nc.compile()
    names = dict(
        xT=xT_d.name,
        cT=cT_d.name,
        wq=wq_d.name,
        wk=wk_d.name,
        wv=wv_d.name,
        wo=wo_d.name,
        out=out_d.name,
    )
    return nc, names


def _get_built():
    if "nc" not in _CACHE:
        _CACHE["nc"], _CACHE["names"] = _build()
    return _CACHE["nc"], _CACHE["names"]


def run(x, context, Wq, Wk, Wv, Wo, bo, trace=False):
    from concourse.bass_utils import run_bass_kernel_spmd

    nc, names = _get_built()
    bf16 = ml_dtypes.bfloat16

    x = np.asarray(x, dtype=np.float32)
    context = np.asarray(context, dtype=np.float32)
    Wq = np.asarray(Wq, dtype=np.float32)
    Wk = np.asarray(Wk, dtype=np.float32)
    Wv = np.asarray(Wv, dtype=np.float32)
    Wo = np.asarray(Wo, dtype=np.float32)
    bo = np.asarray(bo, dtype=np.float32)

    in_maps = []
    for c in range(8):
        b, g = divmod(c, 2)
        cols = slice(g * HC, (g + 1) * HC)
        in_maps.append(
            {
                names["xT"]: np.ascontiguousarray(x[b].T).astype(bf16),
                names["cT"]: np.ascontiguousarray(context[b].T).astype(bf16),
                names["wq"]: np.ascontiguousarray(Wq[:, cols]).astype(bf16),
                names["wk"]: np.ascontiguousarray(Wk[:, cols]).astype(bf16),
                names["wv"]: np.ascontiguousarray(Wv[:, cols]).astype(bf16),
                names["wo"]: np.ascontiguousarray(Wo[cols, :]).astype(bf16),
            }
        )

    res = run_bass_kernel_spmd(
        nc, in_maps, core_ids=list(range(8)), trace=trace,
        stitch_traces=trace,
    )
    out = np.empty((B, N, DIM), dtype=np.float32)
    for b in range(B):
        out[b] = res.results[2 * b][names["out"]] + res.results[2 * b + 1][names["out"]]
    out += bo[None, None, :]
    return out, res


def kernel(x, context, Wq, Wk, Wv, Wo, bo):
    out, _ = run(x, context, Wq, Wk, Wv, Wo, bo, trace=False)
    return out


# revision 53
# speedup vs baseline: 1.0008x; 1.0008x over previous
"""Cross-frame attention kernel for 8 TRN2 NeuronCores.

Sharding: core c handles batch b = c//2 and head-group g = c%2 (4 of the 8
heads).  The host pre-transposes x[b]/context[b] (feature dim onto SBUF
partitions) and casts to bf16; each core computes a partial output
(its 4 heads pushed through the matching Wo rows) and the host sums the
two partials per batch plus the bias.

Device math per core (S^T layout, softmax over the partition j-dim):
  QT = Wq_g^T x^T          [256, 2048]
  KT = Wk_g^T c^T          [256, 2048]
  V  = c Wv_g              [2048, 256] (+ ones column per head)
  S^T = K_h Q_h^T          [j, i] tiles; j-tiles 0..5 run as fp8e4
                           DoubleRow matmuls (2x PE rate, 32+32 d-subtile
                           pairing), j-tiles 6..15 in bf16
  exp via ScalarE (scale=1/8 fused), bf16 out
  O~^T | Z = [V_h|1]^T expS^T   (PSUM accumulate over j)
  A^T = O~^T * bcast(1/Z)  (reciprocal on DVE, partition broadcast on
                           GpSimd/Pool, multiply on DVE)
  out_partial = A^T^T Wo_g [2048, 512] fp32

The schedule is a flat software pipeline over 128 (block, j-group) items:
per item the PE issues the S matmuls first (so the ScalarE exp stream is
never gated by downstream work), then projection / V / normalization /
output-projection fillers, then the AV matmuls of the item two groups
back.  The per-block normalization runs on DVE+Pool only, keeping PE free.

The exp stream on the Scalar engine (16.8M exps/core at ~0.83ns/col) is
the roofline for this kernel; PE is kept just below it via the partial
fp8 S.  fp8 Q/K quantization (after bf16 projection; DVE PSUM->fp8 copy
plus a small SBUF->SBUF DMA shuffle into the [32, 2-dslab, head, seq]
DoubleRow layout) perturbs outputs by ~1.1e-2 rel on 6/16 of the j-range,
inside the 2e-2 tolerance.

Logits are |S/8| <~ 1.1 for this problem's scale, so softmax without
max-subtraction is exact in fp32.
"""

import numpy as np
import ml_dtypes

B = 4
N = 2048  # query length
M = 2048  # context length
DIM = 512
HEADS = 8
DH = 64
HC = 256  # head columns handled per core (4 heads)
P = 128
KO = DIM // P  # 4 k-chunks
NI4 = N // 512  # 4 i-chunks of 512
NJ = M // P  # 16 j-chunks
JPG = 2  # j-chunks per exp group (PSUM banks per S^T buffer)
NG = NJ // JPG  # 8 j-groups per block
FP8_GROUPS = (0, 1, 2, 3)  # j-groups using fp8 DoubleRow (j-tiles 0..7)

_CACHE = {}


def _build():
    from contextlib import ExitStack

    import concourse.mybir as mybir
    import concourse.tile as tile
    from concourse import bacc

    bf = mybir.dt.bfloat16
    f32 = mybir.dt.float32
    fp8 = mybir.dt.float8e4
    Exp = mybir.ActivationFunctionType.Exp
    DR = mybir.MatmulPerfMode.DoubleRow

    nc = bacc.Bacc(None, target_bir_lowering=False, debug=False)
    with tile.TileContext(nc) as tc:
        with ExitStack() as ctx:
            dram = ctx.enter_context(tc.tile_pool(name="dram", bufs=1, space="DRAM"))
            xT_d = dram.tile([DIM, N], bf, kind="ExternalInput")
            cT_d = dram.tile([DIM, M], bf, kind="ExternalInput")
            wq_d = dram.tile([DIM, HC], bf, kind="ExternalInput")
            wk_d = dram.tile([DIM, HC], bf, kind="ExternalInput")
            wv_d = dram.tile([DIM, HC], bf, kind="ExternalInput")
            wo_d = dram.tile([HC, DIM], bf, kind="ExternalInput")
            out_d = dram.tile([N, DIM], f32, kind="ExternalOutput")

            const = ctx.enter_context(tc.tile_pool(name="const", bufs=1))

            xt_sb = const.tile([P, KO, N], bf, tag="xt")
            ct_sb = const.tile([P, KO, M], bf, tag="ct")
            wq_sb = const.tile([P, KO, HC], bf, tag="wq")
            wk_sb = const.tile([P, KO, HC], bf, tag="wk")
            wv_sb = const.tile([P, KO, HC], bf, tag="wv")
            wo_sb = const.tile([P, 2, DIM], bf, tag="wo")
            qT_sb = const.tile([P, 2, N], bf, tag="qT")
            kT_sb = const.tile([P, 2, M], bf, tag="kT")
            # fp8 staging (projection layout) + DoubleRow layout
            # [dmod32, dslab, head, seq]; k8 covers all of M for m=0 (blocks
            # 1-3 run fully fp8) but only j-tiles 0..5 for m=1
            qs8_sb = const.tile([P, 2, N], fp8, tag="qs8")
            ks8_sb = const.tile([P, 2, M], fp8, tag="ks8")
            q8_sb = const.tile([32, 2, 4, N], fp8, tag="q8")
            k8_sb = const.tile([32, 2, 4, M], fp8, tag="k8")
            warm_sb = const.tile([1, 512], bf, tag="warm")
            # all 4 heads' V with a trailing ones column: [j, jo, head, 65]
            vp_sb = const.tile([P, NJ, 4, DH + 1], bf, tag="vp")
            aT_sb = const.tile([P, 2, N], bf, tag="aT")

            dummy_sb = const.tile([1, 1], f32, tag="dummy")
            nc.vector.memset(vp_sb[:, :, :, DH : DH + 1], 1.0)
            nc.vector.memset(warm_sb[:], 0.0)
            # hoist the exp ACT-table load out of the critical path
            nc.scalar.activation(dummy_sb[:], vp_sb[0:1, 0, 0, DH : DH + 1],
                                 Exp, scale=1.0)

            # DMA in, first-needed first, ONE dma_start per piece: the HWDGE
            # queue dispatches serially at ~625ns/DMA, so consolidated
            # transfers keep the preamble's shuffle DMAs near the queue head.
            # The remaining input pieces are issued after the preamble (see
            # below) so they queue behind its shuffle DMAs.
            cT_r = cT_d[:].rearrange("(ko p) i -> p ko i", p=P)
            xT_r = xT_d[:].rearrange("(ko p) i -> p ko i", p=P)
            nc.sync.dma_start(wk_sb[:], wk_d[:].rearrange("(ko p) m -> p ko m", p=P))
            nc.sync.dma_start(ct_sb[:, :, 0:512], cT_r[:, :, 0:512])
            nc.sync.dma_start(wq_sb[:], wq_d[:].rearrange("(ko p) m -> p ko m", p=P))
            nc.sync.dma_start(xt_sb[:, :, 0:512], xT_r[:, :, 0:512])
            nc.sync.dma_start(ct_sb[:, :, 512:1024], cT_r[:, :, 512:1024])
            nc.sync.dma_start(wv_sb[:], wv_d[:].rearrange("(ko p) m -> p ko m", p=P))
            nc.sync.dma_start(ct_sb[:, :, 1024:1536], cT_r[:, :, 1024:1536])
            nc.sync.dma_start(ct_sb[:, :, 1536:2048], cT_r[:, :, 1536:2048])
            nc.sync.dma_start(xt_sb[:, :, 512:1024], xT_r[:, :, 512:1024])

            def dma_xt23_wo():
                nc.sync.dma_start(xt_sb[:, :, 1024:2048], xT_r[:, :, 1024:2048])
                nc.sync.dma_start(
                    wo_sb[:], wo_d[:].rearrange("(r p) n -> p r n", p=P))

            # PSUM budget (8 banks): s 2x2 + o 2x1 + aux 1 + scr 1
            with (
                tc.tile_pool(name="s_ps", bufs=2, space="PSUM") as s_pool,
                tc.tile_pool(name="aux_ps", bufs=1, space="PSUM") as aux_pool,
                tc.tile_pool(name="o_ps", bufs=2, space="PSUM") as o_pool,
                tc.tile_pool(name="scr_ps", bufs=1, space="PSUM") as scr_pool,
                tc.tile_pool(name="e_sb", bufs=14) as e_pool,
                tc.tile_pool(name="small", bufs=2) as small,
                tc.tile_pool(name="ost", bufs=4) as ostp,
            ):
                # ---- building blocks -------------------------------------
                def qk_chunk_pieces(wsb, src_sb, stage8, dst8, bfdst, m, c,
                                    want8, want_bf, pool_tag):
                    """Filler pieces: A/B = 2 ko-matmuls each (B also does
                    the bf16 copy), C = fp8 quantize + DoubleRow shuffle.
                    A chunk's pieces must stay consecutive w.r.t. other
                    tenants of the same PSUM bank."""
                    isl = slice(c * 512, (c + 1) * 512)
                    pool, tag = pool_tag
                    box = []

                    def piece_a():
                        ps = pool.tile([P, 512], f32, tag=tag, name="ps_qk")
                        box.append(ps)
                        for ko in (0, 1):
                            nc.tensor.matmul(
                                ps[:], wsb[:, ko, m * P : (m + 1) * P],
                                src_sb[:, ko, isl],
                                start=(ko == 0), stop=False)

                    def piece_b():
                        ps = box[0]
                        for ko in (2, 3):
                            nc.tensor.matmul(
                                ps[:], wsb[:, ko, m * P : (m + 1) * P],
                                src_sb[:, ko, isl],
                                start=False, stop=(ko == 3))
                        if want_bf:
                            nc.vector.tensor_copy(bfdst[:, m, isl], ps[:])

                    def piece_c():
                        # quantize from the bf16 SBUF copy when available:
                        # the PSUM bank then frees at piece_b, so the next
                        # bank tenant's matmuls don't wait on this copy
                        src = bfdst[:, m, isl] if want_bf else box[0][:]
                        nc.vector.tensor_copy(stage8[:, m, isl], src)
                        for hl in range(2):
                            for s in range(2):
                                pb8 = 64 * hl + 32 * s
                                nc.sync.dma_start(
                                    dst8[:, s, 2 * m + hl, isl],
                                    stage8[pb8 : pb8 + 32, m, isl],
                                )

                    if want8:
                        return piece_a, piece_b, piece_c
                    return piece_a, piece_b

                def kchunk(m, c):
                    # chunk 0 also needs bf16 for m=0 (block 0 runs all-bf16
                    # so its start doesn't wait on the fp8 shuffle DMAs)
                    return qk_chunk_pieces(
                        wk_sb, ct_sb, ks8_sb, k8_sb, kT_sb, m, c,
                        want8=(m == 0 or c < 2), want_bf=(c >= 1 or m == 0),
                        pool_tag=(aux_pool, "aux"))

                def qchunk(m, c):
                    # q-chunks use the scr bank so their PSUM tenancy never
                    # interleaves with k-chunks/vpairs on the aux bank
                    return qk_chunk_pieces(
                        wq_sb, xt_sb, qs8_sb, q8_sb, qT_sb, m, c,
                        want8=True, want_bf=True,
                        pool_tag=(scr_pool, "scr"))

                def vpair_half(g, half):
                    def f():
                        jo = 2 * g + half
                        # alternate banks so each half's matmuls overlap the
                        # previous half's PSUM->SBUF copy instead of WAR-
                        # stalling on the single bank
                        pool, tag = ((aux_pool, "aux") if jo % 2 == 0
                                     else (scr_pool, "scr"))
                        ps = pool.tile([P, HC], f32, tag=tag, name="ps_v")
                        for ko in range(KO):
                            nc.tensor.matmul(
                                ps[:], ct_sb[:, ko, jo * P : (jo + 1) * P],
                                wv_sb[:, ko, :],
                                start=(ko == 0), stop=(ko == KO - 1))
                        nc.vector.tensor_copy(
                            vp_sb[:, jo, :, 0:DH],
                            ps[:].rearrange("p (h d) -> p h d", h=4))
                    return f

                o_tiles = {}
                e_tiles = {}

                def emit_S(bidx, blk, g):
                    i4, m, hl = blk
                    h = 2 * m + hl
                    isl = slice(i4 * 512, (i4 + 1) * 512)
                    s_ps = s_pool.tile([P, JPG, 512], f32, tag="s", name="s_ps")
                    # uniform fp8 fraction across (i, h) so the absmax error
                    # stays even; block 0 g0 stays bf16 (its fp8 staging
                    # isn't shuffled yet at stream start)
                    fp8_here = g in FP8_GROUPS and bidx > 0
                    for jj in range(JPG):
                        j = g * JPG + jj
                        if fp8_here:
                            nc.tensor.matmul(
                                s_ps[:, jj, :],
                                k8_sb[:, :, h, j * P : (j + 1) * P],
                                q8_sb[:, :, h, isl],
                                start=True, stop=True, perf_mode=DR)
                        else:
                            pb = DH * hl
                            nc.tensor.matmul(
                                s_ps[:, jj, :],
                                kT_sb[pb : pb + DH, m, j * P : (j + 1) * P],
                                qT_sb[pb : pb + DH, m, isl],
                                start=True, stop=True)
                    e_sb = e_pool.tile([P, JPG, 512], bf, tag="e", name="e_sb")
                    nc.scalar.activation(e_sb[:], s_ps[:], Exp, scale=0.125)
                    e_tiles[(bidx, g)] = e_sb

                def emit_AV(bidx, blk, g):
                    i4, m, hl = blk
                    h = 2 * m + hl
                    if g == 0:
                        o_tiles[bidx] = o_pool.tile(
                            [DH + 1, 512], f32, tag="o", name="o_ps")
                    o_ps = o_tiles[bidx]
                    e_sb = e_tiles.pop((bidx, g))
                    for jj in range(JPG):
                        j = g * JPG + jj
                        nc.tensor.matmul(
                            o_ps[:], vp_sb[:, j, h, :], e_sb[:, jj, :],
                            start=(j == 0), stop=(j == NJ - 1))

                def finish(bidx, blk):
                    def f():
                        i4, m, hl = blk
                        pb = DH * hl
                        isl = slice(i4 * 512, (i4 + 1) * 512)
                        o_ps = o_tiles.pop(bidx)
                        rz = small.tile([1, 512], f32, tag="rz", name="rz")
                        nc.vector.reciprocal(rz[:], o_ps[DH : DH + 1, :])
                        rbc = small.tile([DH, 512], f32, tag="rbc", name="rbc")
                        nc.gpsimd.partition_broadcast(rbc[:], rz[0:1, :])
                        nc.vector.tensor_mul(
                            aT_sb[pb : pb + DH, m, isl], o_ps[0:DH, :], rbc[:])
                    return f

                def wo_piece(i4, ii, tail=False):
                    def f():
                        i = i4 * 4 + ii
                        pool, tag = ((scr_pool, "scr") if ii % 2 == 0
                                     else (aux_pool, "aux"))
                        ps = pool.tile([P, DIM], f32, tag=tag, name="p3_ps")
                        for m in range(2):
                            nc.tensor.matmul(
                                ps[:], aT_sb[:, m, i * P : (i + 1) * P],
                                wo_sb[:, m, :],
                                start=(m == 0), stop=(m == 1))
                        ost = ostp.tile([P, DIM], f32, tag="ost", name="ost")
                        if tail and ii % 2 == 1:
                            # post-stream: ScalarE is idle, split the copies
                            nc.scalar.copy(ost[:], ps[:])
                        else:
                            nc.vector.tensor_copy(ost[:], ps[:])
                        nc.sync.dma_start(out_d[i * P : (i + 1) * P, :], ost[:])
                    return f

                # ---- schedule --------------------------------------------
                blocks = [
                    (0, 0, 0), (0, 0, 1), (1, 0, 0), (1, 0, 1),
                    (0, 1, 0), (0, 1, 1), (2, 0, 0), (2, 0, 1),
                    (1, 1, 0), (1, 1, 1), (3, 0, 0), (3, 0, 1),
                    (2, 1, 0), (2, 1, 1), (3, 1, 0), (3, 1, 1),
                ]

                fillers = {}

                def add(bidx, g, *fs):
                    fillers.setdefault((bidx, g), []).extend(fs)

                # block 0 j-groups run all-bf16 and its AVs lag 8 groups, so
                # the 8 V-pair projections spread over blocks 0-1.  All aux-
                # bank tenants (k-chunks, vpairs) are laid out strictly
                # sequentially: a tenant's pieces finish before the next
                # tenant's first piece.
                add(0, 0, *kchunk(0, 1))            # bf16 by S(0,2); fp8 by blk1 g2
                add(0, 2, *kchunk(0, 2))            # by S(0,4)
                add(0, 4, *kchunk(0, 3))            # by S(0,6)
                for k in range(8):
                    b_at, g_at = divmod(7 + k, NG)  # vp0 (0,7) ... vp7 (1,6)
                    add(b_at, g_at, vpair_half(k, 0), vpair_half(k, 1))
                for (m, c), (b_at, g_at) in {       # aux tenants, sequential
                    (1, 0): (2, 0),   # k8 m=1 used from block 4 (fp8)
                    (1, 1): (2, 2),
                    (1, 2): (3, 2),   # kT bf16 m=1 groups 4+
                    (1, 3): (3, 4),
                }.items():
                    add(b_at, g_at, *kchunk(m, c))
                for (m, c), (b_at, g_at) in {       # scr tenants, sequential
                    (0, 1): (0, 5),   # qT i4=1 used from block 2
                    (1, 0): (3, 0),   # q m=1 used from block 4
                    (0, 2): (4, 0),   # i4=2 used from block 6
                    (1, 1): (5, 0),   # m=1 i4=1 used from block 8
                    (0, 3): (7, 0),   # i4=3 used from block 10
                    (1, 2): (9, 0),   # used from block 12
                    (1, 3): (11, 0),  # used from block 14
                }.items():
                    pieces = qchunk(m, c)
                    add(b_at, g_at, pieces[0])
                    add(b_at, g_at + 1, *pieces[1:])
                # The AV lag stays at 8 through the first three blocks
                # (absorbing the V-projection PE deficit) and then decays to
                # the steady-state 2; the last block drops to 1 so the tail
                # drains fast.
                def lag_at(gidx, bidx):
                    if bidx == 15:
                        return 1
                    if gidx < 32:
                        return 12
                    return max(2, 12 - (gidx - 31) // 3)

                # dry-run the lag queue to learn when each AV is emitted,
                # then place each block's normalization right after its last
                # AV and each wo after its four blocks' normalizations
                pop_slot = {}
                dq = []
                for b in range(16):
                    for g in range(NG):
                        gidx = b * NG + g
                        dq.append((b, g))
                        while len(dq) > lag_at(gidx, b):
                            pop_slot[dq.pop(0)] = gidx
                for item in dq:
                    pop_slot[item] = 16 * NG
                fin_slot = {}
                for b in range(16):
                    fin_slot[b] = pop_slot[(b, 7)] + 1
                    if fin_slot[b] < 16 * NG:
                        add(*divmod(fin_slot[b], NG), finish(b, blocks[b]))
                for i4 in range(4):
                    s0 = max(fin_slot[b] for b, blk in enumerate(blocks)
                             if blk[0] == i4) + 1
                    if s0 + 6 < 16 * NG:
                        for ii in range(4):
                            add(*divmod(s0 + 2 * ii, NG), wo_piece(i4, ii))

                # PE warm-up: keep the tensor engine busy from t~0.7us so
                # the p-state ramp (full speed only after 3us of continuous
                # execution) completes before the first real projections.
                warm_ps = aux_pool.tile([P, 512], f32, tag="aux", name="warm")
                for _ in range(9):
                    nc.tensor.matmul(warm_ps[0:1, :], warm_sb[0:1, 0:1],
                                     warm_sb[:], start=True, stop=True)

                # preamble: the chunk-0 projections feeding the first block
                # (k on aux, q on scr so they overlap), fp8 staging last so
                # the bf16 copies feeding S(0,0) finish first; remaining
                # input DMAs queue behind the preamble shuffle DMAs.
                k00 = kchunk(0, 0)
                q00 = qchunk(0, 0)
                k00[0](); k00[1]()
                q00[0](); q00[1]()
                k00[2](); q00[2]()
                # bulk input DMAs are emitted as late fillers so the early
                # fp8-shuffle DMAs aren't stuck behind them in the queue
                add(1, 7, dma_xt23_wo)

                # main loop: S first, then fillers, then lagged AV; the AV
                # lag ramps 8 -> 2 across blocks 1-2 (absorbing block 0's
                # V-projection PE deficit) and drops to 1 for the last block
                av_q = []
                for bidx, blk in enumerate(blocks):
                    for g in range(NG):
                        gidx = bidx * NG + g
                        emit_S(bidx, blk, g)
                        for f in fillers.get((bidx, g), []):
                            f()
                        av_q.append((bidx, blk, g))
                        while len(av_q) > lag_at(gidx, bidx):
                            emit_AV(*av_q.pop(0))
                while av_q:
                    emit_AV(*av_q.pop(0))
                for b in range(16):
                    if fin_slot[b] >= 16 * NG:
                        finish(b, blocks[b])()
                for i4 in range(4):
                    s0 = max(fin_slot[b] for b, blk in enumerate(blocks)
                             if blk[0] == i4) + 1
                    if s0 + 3 >= 16 * NG:
                        for ii in range(4):
                            wo_piece(i4, ii, tail=True)()

    nc.compile()
    names = dict(
        xT=xT_d.name,
        cT=cT_d.name,
        wq=wq_d.name,
        wk=wk_d.name,
        wv=wv_d.name,
        wo=wo_d.name,
        out=out_d.name,
    )
    return nc, names


def _get_built():
    if "nc" not in _CACHE:
        _CACHE["nc"], _CACHE["names"] = _build()
    return _CACHE["nc"], _CACHE["names"]


def run(x, context, Wq, Wk, Wv, Wo, bo, trace=False):
    from concourse.bass_utils import run_bass_kernel_spmd

    nc, names = _get_built()
    bf16 = ml_dtypes.bfloat16

    x = np.asarray(x, dtype=np.float32)
    context = np.asarray(context, dtype=np.float32)
    Wq = np.asarray(Wq, dtype=np.float32)
    Wk = np.asarray(Wk, dtype=np.float32)
    Wv = np.asarray(Wv, dtype=np.float32)
    Wo = np.asarray(Wo, dtype=np.float32)
    bo = np.asarray(bo, dtype=np.float32)

    in_maps = []
    for c in range(8):
        b, g = divmod(c, 2)
        cols = slice(g * HC, (g + 1) * HC)
        in_maps.append(
            {
                names["xT"]: np.ascontiguousarray(x[b].T).astype(bf16),
                names["cT"]: np.ascontiguousarray(context[b].T).astype(bf16),
                names["wq"]: np.ascontiguousarray(Wq[:, cols]).astype(bf16),
                names["wk"]: np.ascontiguousarray(Wk[:, cols]).astype(bf16),
                names["wv"]: np.ascontiguousarray(Wv[:, cols]).astype(bf16),
                names["wo"]: np.ascontiguousarray(Wo[cols, :]).astype(bf16),
            }
        )

    res = run_bass_kernel_spmd(
        nc, in_maps, core_ids=list(range(8)), trace=trace,
        stitch_traces=trace,
    )
    out = np.empty((B, N, DIM), dtype=np.float32)
    for b in range(B):
        out[b] = res.results[2 * b][names["out"]] + res.results[2 * b + 1][names["out"]]
    out += bo[None, None, :]
    return out, res


def kernel(x, context, Wq, Wk, Wv, Wo, bo):
    out, _ = run(x, context, Wq, Wk, Wv, Wo, bo, trace=False)
    return out
